# revision 22
# baseline (speedup 1.0000x reference)
"""Trainium2 Bass kernel for the NiN-Conv2D problem.

Network: per-pixel MLP over 7x7x3 patches, independent per filter f:
  h0 = relu(P @ W0[:,:,f] + b0)   (147 -> 32)
  h1 = relu(h0 @ W1[:,:,f] + b1)  (32 -> 16)
  out = relu(h1 @ W2[:,:,f] + b2) (16 -> 1)
for B=32, H=W=32, F=128.

Strategy: data-parallel over batch across 8 NeuronCores (4 images each).
On each core everything runs in a "feature-major" orientation: activations
live as (d*f on partitions, pixels on free dim), weights are the stationary
matmul operand, so no transposes are needed between layers.

  L0: per group of 4 filters, lhsT = W0 chunk (K=128 / K=19 accumulated),
      rhs = im2col-transposed patches (K, pix) -> PSUM (128=4f*32, pix)
  L1: per pair of groups, block-diag W1 (128, 64), two matmuls fill the
      two partition halves of one PSUM tile -> (128=8f*16, pix)
  L2: per 32-filter block, 4 accumulating block-diag matmuls -> (f, pix)

Bias+ReLU+cast(PSUM->SBUF) fused into one ACT/DVE op, split across both
engines to balance load. Matmul operands are bf16 (fp32 PSUM accumulate).

The PE cost of a matmul is ~N_free cycles regardless of K/M, so this
instruction mix sits at the bf16 structural floor; the Tile scheduler's
emergent interleaving of L1/L2 into L0 keeps the PE ~80% busy. Startup
DMAs are fine-sliced and triggered from both the sync and scalar queues
(each trigger costs ~600ns of one engine's queue) so the PE starts ~4us
earlier than with one serial trigger queue.

Host side: im2col transpose + weight packing (pure layout, no FLOPs).
"""
import numpy as np
import ml_dtypes

import concourse.bass as bass
import concourse.mybir as mybir
from concourse import bacc, tile
from concourse import bass_utils
from concourse.bass import ts

KH, KW = 7, 7
B, H, W, C, F = 32, 32, 32, 3, 128
K, D0, D1 = 147, 32, 16
NCORES = 8
BPC = B // NCORES            # 4 images per core
NPIX = BPC * H * W           # 4096 pixels per core
PTILE = 512
NT = NPIX // PTILE           # 8 pixel tiles

BF16 = mybir.dt.bfloat16
F32 = mybir.dt.float32
NPBF16 = ml_dtypes.bfloat16


# ----------------------------------------------------------------------------
# host-side packing (layout only)
# ----------------------------------------------------------------------------

def _pack_weights(w0, b0, w1, b1, w2, b2):
    """Shared (core-independent) weight/bias packing. Returns dict of np arrays."""
    w0 = np.asarray(w0, np.float32)
    w1 = np.asarray(w1, np.float32)
    w2 = np.asarray(w2, np.float32)
    b0 = np.asarray(b0, np.float32)
    b1 = np.asarray(b1, np.float32)
    b2 = np.asarray(b2, np.float32)

    w0a = np.empty((128, 32, 128), np.float32)   # [k, group, m=fl*32+d]
    # chunk2 (K rows 128..146 + bias row) packed for 4-way row-tiled
    # concurrency: group g lives at partitions 32*(g%4)+k, cols g*128+m.
    # Row 32*(g%4)+19 carries b0 (the patch tile has ones there), so the
    # PSUM result already includes the bias and the relu op needs none.
    w0b = np.zeros((128, 32, 128), np.float32)
    for g in range(32):
        m = w0[:, :, 4 * g:4 * g + 4].transpose(0, 2, 1).reshape(K, 128)
        w0a[:, g, :] = m[:128]
        r = g % 4
        w0b[32 * r:32 * r + 19, g, :] = m[128:]
        w0b[32 * r + 19, g, :] = b0[:, 4 * g:4 * g + 4].T.reshape(128)

    w1bd = np.zeros((128, 32, 64), np.float32)   # [k=fl*32+d0, g, m=fl*16+d1]
    b1s = np.empty((128, 16), np.float32)
    for g in range(32):
        for fl in range(4):
            f = 4 * g + fl
            w1bd[fl * 32:(fl + 1) * 32, g, fl * 16:(fl + 1) * 16] = w1[:, :, f]
    for p in range(16):
        for half in range(2):
            g = 2 * p + half
            b1s[half * 64:(half + 1) * 64, p] = b1[:, 4 * g:4 * g + 4].T.reshape(64)

    w2bd = np.zeros((128, 16, 32), np.float32)   # [k=half*64+fl*16+d1, pair, col]
    for p in range(16):
        for half in range(2):
            for fl in range(4):
                f = 8 * p + half * 4 + fl
                col = f - 32 * (p // 4)
                w2bd[half * 64 + fl * 16:half * 64 + (fl + 1) * 16, p, col] = w2[:, 0, f]
    b2s = b2.reshape(128, 1).astype(np.float32)

    return {
        "w0a": w0a.reshape(128, 4096).astype(NPBF16),
        "w0b": w0b.reshape(128, 4096).astype(NPBF16),
        "w1bd": w1bd.reshape(128, 2048).astype(NPBF16),
        "w2bd": w2bd.reshape(128, 512).astype(NPBF16),
        "b1s": b1s, "b2s": b2s,
    }


def _im2col_T(x_core):
    """x_core (4,32,32,3) fp32 -> PT (147, 4096) with k=(kh*7+kw)*3+c."""
    xp = np.pad(np.asarray(x_core, np.float32), ((0, 0), (3, 3), (3, 3), (0, 0)))
    PT = np.empty((K, NPIX), np.float32)
    for kh in range(KH):
        for kw in range(KW):
            blk = xp[:, kh:kh + H, kw:kw + W, :]
            t = kh * 7 + kw
            PT[t * 3:t * 3 + 3] = blk.transpose(3, 0, 1, 2).reshape(3, NPIX)
    return PT


# ----------------------------------------------------------------------------
# device kernel
# ----------------------------------------------------------------------------

def _body(tc):
    nc = tc.nc
    Relu = mybir.ActivationFunctionType.Relu
    Add, Max = mybir.AluOpType.add, mybir.AluOpType.max

    pt1 = nc.dram_tensor("pt1", [128, NPIX], BF16, kind="ExternalInput").ap()
    pt2 = nc.dram_tensor("pt2", [128, NPIX], BF16, kind="ExternalInput").ap()
    w0a = nc.dram_tensor("w0a", [128, 4096], BF16, kind="ExternalInput").ap()
    w0b = nc.dram_tensor("w0b", [128, 4096], BF16, kind="ExternalInput").ap()
    w1bd = nc.dram_tensor("w1bd", [128, 2048], BF16, kind="ExternalInput").ap()
    w2bd = nc.dram_tensor("w2bd", [128, 512], BF16, kind="ExternalInput").ap()
    b1d = nc.dram_tensor("b1s", [128, 16], F32, kind="ExternalInput").ap()
    b2d = nc.dram_tensor("b2s", [128, 1], F32, kind="ExternalInput").ap()
    out = nc.dram_tensor("out", [128, NPIX], BF16, kind="ExternalOutput").ap()

    with (
        tc.tile_pool(name="consts", bufs=1) as cpool,
        tc.tile_pool(name="h0", bufs=20) as h0pool,
        tc.tile_pool(name="h1", bufs=20) as h1pool,
        tc.tile_pool(name="outs", bufs=3) as opool,
        tc.tile_pool(name="l0p", bufs=3, space="PSUM") as l0pool,
        tc.tile_pool(name="l12p", bufs=2, space="PSUM") as l12pool,
    ):
        # ---- input staging.  Transfers are fine-sliced and triggered in
        # strict first-use order, alternating the sync/scalar trigger
        # queues, so the first quad's 4 slices (~0.5MB) land without
        # contending with later weight traffic and the PE starts ~4us
        # after the preamble instead of ~6.
        was = [cpool.tile([128, 512], BF16, name=f"w0a{i}", tag=f"w0a{i}")
               for i in range(8)]
        wbs = [cpool.tile([128, 512], BF16, name=f"w0b{i}", tag=f"w0b{i}")
               for i in range(8)]
        pt1s = [cpool.tile([128, PTILE], BF16, name=f"pt1_{t}", tag=f"pt1_{t}")
                for t in range(NT)]
        pt2s = [cpool.tile([128, PTILE], BF16, name=f"pt2_{t}", tag=f"pt2_{t}")
                for t in range(NT)]
        w1s = cpool.tile([128, 2048], BF16, name="w1", tag="w1")
        w2s = cpool.tile([128, 512], BF16, name="w2", tag="w2")
        b1s = cpool.tile([128, 16], F32, name="b1", tag="b1")
        b2s = cpool.tile([128, 1], F32, name="b2", tag="b2")

        nc.sync.dma_start(was[0][:], w0a[:, 0:512])
        nc.scalar.dma_start(pt1s[0][:], pt1[:, 0:PTILE])
        nc.scalar.dma_start(pt2s[0][:], pt2[:, 0:PTILE])
        nc.sync.dma_start(wbs[0][:], w0b[:, 0:512])
        nc.sync.dma_start(was[1][:], w0a[:, 512:1024])
        nc.scalar.dma_start(wbs[1][:], w0b[:, 512:1024])
        nc.scalar.dma_start(w1s[:], w1bd)
        nc.sync.dma_start(was[2][:], w0a[:, 1024:1536])
        nc.sync.dma_start(wbs[2][:], w0b[:, 1024:1536])
        nc.scalar.dma_start(b1s[:], b1d)
        nc.sync.dma_start(was[3][:], w0a[:, 1536:2048])
        nc.sync.dma_start(wbs[3][:], w0b[:, 1536:2048])
        nc.scalar.dma_start(w2s[:], w2bd)
        for i in range(4, 8):
            nc.sync.dma_start(was[i][:], w0a[:, ts(i, 512)])
            nc.sync.dma_start(wbs[i][:], w0b[:, ts(i, 512)])
        nc.sync.dma_start(b2s[:], b2d)
        for t in range(1, NT):
            nc.sync.dma_start(pt1s[t][:], pt1[:, ts(t, PTILE)])
            nc.sync.dma_start(pt2s[t][:], pt2[:, ts(t, PTILE)])

        def relu(dst, src, bias, idx):
            # alternate whole tiles between ScalarE and VectorE
            if idx % 2 == 0:
                nc.scalar.activation(dst, src, Relu, bias=bias)
            else:
                nc.vector.tensor_scalar(dst, src, bias, 0.0, Add, Max)

        for t in range(NT):
            pix = ts(t, PTILE)
            # ---- layer 0: 8 quads of 4 filter-groups; two (128,1024) PSUM
            # tiles per quad (2 groups each, one per column half); chunk2
            # (K rows 128..146 + bias row) runs 4-way concurrent via
            # row-group tiling. Bias rides in the matmul, so one wide
            # bias-free relu op covers a whole tile.
            h0 = []       # 16 tiles (128,1024): groups (2j, 2j+1)
            for q in range(8):
                psA = l0pool.tile([128, 2 * PTILE], F32, tag="l0")
                psB = l0pool.tile([128, 2 * PTILE], F32, tag="l0")
                for r in range(4):
                    g = 4 * q + r
                    ps = psA if r < 2 else psB
                    dst = ps[:, ts(r % 2, PTILE)]
                    nc.tensor.matmul(dst, was[g // 4][:, ts(g % 4, 128)],
                                     pt1s[t][:], start=True, stop=False)
                for r in range(4):
                    g = 4 * q + r
                    ps = psA if r < 2 else psB
                    dst = ps[:, ts(r % 2, PTILE)]
                    nc.tensor.matmul(dst,
                                     wbs[g // 4][32 * r:32 * r + 20, ts(g % 4, 128)],
                                     pt2s[t][32 * r:32 * r + 20, :],
                                     start=False, stop=True,
                                     tile_position=(32 * r, 0))
                # drain each PSUM tile with two parallel half-acts, one per
                # engine, to cut the buffer-recycle and h0-ready latency.
                # DVE (slower) takes the earlier-finished left half.
                # Each PSUM half goes to its OWN h0 tile: L1's two
                # column-grouped matmuls then stream rhs from two distinct
                # SBUF tiles, which lets them overlap on the PE the way
                # L2's four-way column-tiled matmuls (4 distinct h1 rhs
                # tiles) demonstrably do.
                for j, ps in ((2 * q, psA), (2 * q + 1, psB)):
                    ha = h0pool.tile([128, PTILE], BF16, name="ha", tag="h0")
                    hb = h0pool.tile([128, PTILE], BF16, name="hb", tag="h0")
                    nc.vector.tensor_scalar_max(ha[:], ps[:, 0:PTILE], 0.0)
                    nc.scalar.activation(hb[:], ps[:, PTILE:], Relu)
                    h0.append((ha, hb))
            # ---- layer 1: 16 pairs of groups -> (128 = 8f*16, pix)
            h1 = []
            for p in range(16):
                ps = l12pool.tile([128, PTILE], F32, tag="l12")
                ha, hb = h0[p]
                nc.tensor.matmul(ps[0:64, :], w1s[:, ts(2 * p, 64)],
                                 ha[:], start=True, stop=True)
                nc.tensor.matmul(ps[64:128, :], w1s[:, ts(2 * p + 1, 64)],
                                 hb[:], start=True, stop=True)
                h = h1pool.tile([128, PTILE], BF16, tag="h1")
                relu(h[:], ps[:], b1s[:, p:p + 1], p)
                h1.append(h)
            # ---- layer 2: 4 blocks of 32 filters; q-major order so the 4
            # blocks' matmuls hit disjoint PE column groups concurrently
            ps2 = l12pool.tile([128, PTILE], F32, tag="l12")
            for q in range(4):
                for jj in range(4):
                    p = 4 * jj + q
                    nc.tensor.matmul(ps2[32 * jj:32 * jj + 32, :],
                                     w2s[:, ts(p, 32)], h1[p][:],
                                     start=(q == 0), stop=(q == 3),
                                     tile_position=(0, 32 * jj))
            ot = opool.tile([128, PTILE], BF16, tag="o")
            nc.scalar.activation(ot[:], ps2[:], Relu, bias=b2s[:, 0:1])
            nc.sync.dma_start(out[:, pix], ot[:])


_COMPILED = None


def _get_compiled():
    global _COMPILED
    if _COMPILED is None:
        import time as _time
        t0 = _time.time()
        nc = bacc.Bacc("TRN2", target_bir_lowering=False, debug=False,
                       num_devices=NCORES)
        with tile.TileContext(nc) as tc:
            _body(tc)
        t1 = _time.time()
        nc.compile()
        t2 = _time.time()
        print(f"[kernel] tile build+schedule {t1 - t0:.1f}s, bacc compile {t2 - t1:.1f}s",
              flush=True)
        _COMPILED = nc
    return _COMPILED


# ----------------------------------------------------------------------------
# public entry point
# ----------------------------------------------------------------------------

def kernel(x, w0, b0, w1, b1, w2, b2, _trace=False):
    x = np.asarray(x, np.float32)
    shared = _pack_weights(w0, b0, w1, b1, w2, b2)

    in_maps = []
    for k in range(NCORES):
        PT = _im2col_T(x[BPC * k:BPC * (k + 1)])
        m = dict(shared)
        m["pt1"] = PT[:128].astype(NPBF16)
        # chunk2 rows replicated at partitions 32r (4-way row tiling),
        # with a ones row at 32r+19 that carries b0 through the matmul
        pt2 = np.zeros((128, NPIX), np.float32)
        for r in range(4):
            pt2[32 * r:32 * r + 19] = PT[128:]
            pt2[32 * r + 19] = 1.0
        m["pt2"] = pt2.astype(NPBF16)
        in_maps.append(m)

    import time as _time
    nc = _get_compiled()
    t0 = _time.time()
    res = bass_utils.run_bass_kernel_spmd(
        nc, in_maps, core_ids=list(range(NCORES)), trace=_trace)
    print(f"[kernel] run_bass_kernel_spmd {_time.time() - t0:.1f}s", flush=True)

    outs = []
    for k in range(NCORES):
        oc = np.asarray(res.results[k]["out"], dtype=np.float32)  # (128, 4096)
        outs.append(oc.reshape(F, BPC, H, W).transpose(1, 2, 3, 0))
    full = np.concatenate(outs, axis=0).astype(np.float32)
    if _trace:
        return full, res
    return full


# revision 23
# speedup vs baseline: 1.0254x; 1.0254x over previous
"""Trainium2 Bass kernel for the NiN-Conv2D problem.

Network: per-pixel MLP over 7x7x3 patches, independent per filter f:
  h0 = relu(P @ W0[:,:,f] + b0)   (147 -> 32)
  h1 = relu(h0 @ W1[:,:,f] + b1)  (32 -> 16)
  out = relu(h1 @ W2[:,:,f] + b2) (16 -> 1)
for B=32, H=W=32, F=128.

Strategy: data-parallel over batch across 8 NeuronCores (4 images each).
On each core everything runs in a "feature-major" orientation: activations
live as (d*f on partitions, pixels on free dim), weights are the stationary
matmul operand, so no transposes are needed between layers.

  L0: per group of 4 filters, lhsT = W0 chunk (K=128 / K=19 accumulated),
      rhs = im2col-transposed patches (K, pix) -> PSUM (128=4f*32, pix)
  L1: per pair of groups, block-diag W1 (128, 64), two matmuls fill the
      two partition halves of one PSUM tile -> (128=8f*16, pix)
  L2: per 32-filter block, 4 accumulating block-diag matmuls -> (f, pix)

Bias+ReLU+cast(PSUM->SBUF) fused into one ACT/DVE op, split across both
engines to balance load. Matmul operands are bf16 (fp32 PSUM accumulate).

The PE cost of a matmul is ~N_free cycles regardless of K/M, so this
instruction mix sits at the bf16 structural floor; the Tile scheduler's
emergent interleaving of L1/L2 into L0 keeps the PE ~80% busy. Startup
DMAs are fine-sliced and triggered from both the sync and scalar queues
(each trigger costs ~600ns of one engine's queue) so the PE starts ~4us
earlier than with one serial trigger queue.

Host side: im2col transpose + weight packing (pure layout, no FLOPs).
"""
import numpy as np
import ml_dtypes

import concourse.bass as bass
import concourse.mybir as mybir
from concourse import bacc, tile
from concourse import bass_utils
from concourse.bass import ts

KH, KW = 7, 7
B, H, W, C, F = 32, 32, 32, 3, 128
K, D0, D1 = 147, 32, 16
NCORES = 8
BPC = B // NCORES            # 4 images per core
NPIX = BPC * H * W           # 4096 pixels per core
PTILE = 512
NT = NPIX // PTILE           # 8 pixel tiles

BF16 = mybir.dt.bfloat16
F32 = mybir.dt.float32
NPBF16 = ml_dtypes.bfloat16


# ----------------------------------------------------------------------------
# host-side packing (layout only)
# ----------------------------------------------------------------------------

def _pack_weights(w0, b0, w1, b1, w2, b2):
    """Shared (core-independent) weight/bias packing. Returns dict of np arrays."""
    w0 = np.asarray(w0, np.float32)
    w1 = np.asarray(w1, np.float32)
    w2 = np.asarray(w2, np.float32)
    b0 = np.asarray(b0, np.float32)
    b1 = np.asarray(b1, np.float32)
    b2 = np.asarray(b2, np.float32)

    w0a = np.empty((128, 32, 128), np.float32)   # [k, group, m=fl*32+d]
    # chunk2 (K rows 128..146 + bias row) packed for 4-way row-tiled
    # concurrency: group g lives at partitions 32*(g%4)+k, cols g*128+m.
    # Row 32*(g%4)+19 carries b0 (the patch tile has ones there), so the
    # PSUM result already includes the bias and the relu op needs none.
    w0b = np.zeros((128, 32, 128), np.float32)
    for g in range(32):
        m = w0[:, :, 4 * g:4 * g + 4].transpose(0, 2, 1).reshape(K, 128)
        w0a[:, g, :] = m[:128]
        r = g % 4
        w0b[32 * r:32 * r + 19, g, :] = m[128:]
        w0b[32 * r + 19, g, :] = b0[:, 4 * g:4 * g + 4].T.reshape(128)

    w1bd = np.zeros((128, 32, 64), np.float32)   # [k=fl*32+d0, g, m=fl*16+d1]
    b1s = np.empty((128, 16), np.float32)
    for g in range(32):
        for fl in range(4):
            f = 4 * g + fl
            w1bd[fl * 32:(fl + 1) * 32, g, fl * 16:(fl + 1) * 16] = w1[:, :, f]
    for p in range(16):
        for half in range(2):
            g = 2 * p + half
            b1s[half * 64:(half + 1) * 64, p] = b1[:, 4 * g:4 * g + 4].T.reshape(64)

    w2bd = np.zeros((128, 16, 32), np.float32)   # [k=half*64+fl*16+d1, pair, col]
    for p in range(16):
        for half in range(2):
            for fl in range(4):
                f = 8 * p + half * 4 + fl
                col = f - 32 * (p // 4)
                w2bd[half * 64 + fl * 16:half * 64 + (fl + 1) * 16, p, col] = w2[:, 0, f]
    b2s = b2.reshape(128, 1).astype(np.float32)

    return {
        "w0a": w0a.reshape(128, 4096).astype(NPBF16),
        "w0b": w0b.reshape(128, 4096).astype(NPBF16),
        "w1bd": w1bd.reshape(128, 2048).astype(NPBF16),
        "w2bd": w2bd.reshape(128, 512).astype(NPBF16),
        "b1s": b1s, "b2s": b2s,
    }


def _im2col_T(x_core):
    """x_core (4,32,32,3) fp32 -> PT (147, 4096) with k=(kh*7+kw)*3+c."""
    xp = np.pad(np.asarray(x_core, np.float32), ((0, 0), (3, 3), (3, 3), (0, 0)))
    PT = np.empty((K, NPIX), np.float32)
    for kh in range(KH):
        for kw in range(KW):
            blk = xp[:, kh:kh + H, kw:kw + W, :]
            t = kh * 7 + kw
            PT[t * 3:t * 3 + 3] = blk.transpose(3, 0, 1, 2).reshape(3, NPIX)
    return PT


# ----------------------------------------------------------------------------
# device kernel
# ----------------------------------------------------------------------------

def _body(tc):
    nc = tc.nc
    Relu = mybir.ActivationFunctionType.Relu
    Add, Max = mybir.AluOpType.add, mybir.AluOpType.max

    pt1 = nc.dram_tensor("pt1", [128, NPIX], BF16, kind="ExternalInput").ap()
    pt2 = nc.dram_tensor("pt2", [128, NPIX], BF16, kind="ExternalInput").ap()
    w0a = nc.dram_tensor("w0a", [128, 4096], BF16, kind="ExternalInput").ap()
    w0b = nc.dram_tensor("w0b", [128, 4096], BF16, kind="ExternalInput").ap()
    w1bd = nc.dram_tensor("w1bd", [128, 2048], BF16, kind="ExternalInput").ap()
    w2bd = nc.dram_tensor("w2bd", [128, 512], BF16, kind="ExternalInput").ap()
    b1d = nc.dram_tensor("b1s", [128, 16], F32, kind="ExternalInput").ap()
    b2d = nc.dram_tensor("b2s", [128, 1], F32, kind="ExternalInput").ap()
    out = nc.dram_tensor("out", [128, NPIX], BF16, kind="ExternalOutput").ap()

    with (
        tc.tile_pool(name="consts", bufs=1) as cpool,
        tc.tile_pool(name="h0", bufs=40) as h0pool,
        tc.tile_pool(name="h1", bufs=20) as h1pool,
        tc.tile_pool(name="outs", bufs=3) as opool,
        tc.tile_pool(name="l0p", bufs=3, space="PSUM") as l0pool,
        tc.tile_pool(name="l12p", bufs=2, space="PSUM") as l12pool,
    ):
        # ---- input staging.  Transfers are fine-sliced and triggered in
        # strict first-use order, alternating the sync/scalar trigger
        # queues, so the first quad's 4 slices (~0.5MB) land without
        # contending with later weight traffic and the PE starts ~4us
        # after the preamble instead of ~6.
        was = [cpool.tile([128, 512], BF16, name=f"w0a{i}", tag=f"w0a{i}")
               for i in range(8)]
        wbs = [cpool.tile([128, 512], BF16, name=f"w0b{i}", tag=f"w0b{i}")
               for i in range(8)]
        pt1s = [cpool.tile([128, PTILE], BF16, name=f"pt1_{t}", tag=f"pt1_{t}")
                for t in range(NT)]
        pt2s = [cpool.tile([128, PTILE], BF16, name=f"pt2_{t}", tag=f"pt2_{t}")
                for t in range(NT)]
        w1s = cpool.tile([128, 2048], BF16, name="w1", tag="w1")
        w2s = cpool.tile([128, 512], BF16, name="w2", tag="w2")
        b1s = cpool.tile([128, 16], F32, name="b1", tag="b1")
        b2s = cpool.tile([128, 1], F32, name="b2", tag="b2")

        nc.sync.dma_start(was[0][:], w0a[:, 0:512])
        nc.scalar.dma_start(pt1s[0][:], pt1[:, 0:PTILE])
        nc.scalar.dma_start(pt2s[0][:], pt2[:, 0:PTILE])
        nc.sync.dma_start(wbs[0][:], w0b[:, 0:512])
        nc.sync.dma_start(was[1][:], w0a[:, 512:1024])
        nc.scalar.dma_start(wbs[1][:], w0b[:, 512:1024])
        nc.scalar.dma_start(w1s[:], w1bd)
        nc.sync.dma_start(was[2][:], w0a[:, 1024:1536])
        nc.sync.dma_start(wbs[2][:], w0b[:, 1024:1536])
        nc.scalar.dma_start(b1s[:], b1d)
        nc.sync.dma_start(was[3][:], w0a[:, 1536:2048])
        nc.sync.dma_start(wbs[3][:], w0b[:, 1536:2048])
        nc.scalar.dma_start(w2s[:], w2bd)
        for i in range(4, 8):
            nc.sync.dma_start(was[i][:], w0a[:, ts(i, 512)])
            nc.sync.dma_start(wbs[i][:], w0b[:, ts(i, 512)])
        nc.sync.dma_start(b2s[:], b2d)
        for t in range(1, NT):
            nc.sync.dma_start(pt1s[t][:], pt1[:, ts(t, PTILE)])
            nc.sync.dma_start(pt2s[t][:], pt2[:, ts(t, PTILE)])

        def relu(dst, src, bias, idx):
            # alternate whole tiles between ScalarE and VectorE
            if idx % 2 == 0:
                nc.scalar.activation(dst, src, Relu, bias=bias)
            else:
                nc.vector.tensor_scalar(dst, src, bias, 0.0, Add, Max)

        for t in range(NT):
            pix = ts(t, PTILE)
            # ---- layer 0: 8 quads of 4 filter-groups; two (128,1024) PSUM
            # tiles per quad (2 groups each, one per column half); chunk2
            # (K rows 128..146 + bias row) runs 4-way concurrent via
            # row-group tiling. Bias rides in the matmul, so one wide
            # bias-free relu op covers a whole tile.
            h0 = []       # 16 tiles (128,1024): groups (2j, 2j+1)
            for q in range(8):
                psA = l0pool.tile([128, 2 * PTILE], F32, tag="l0")
                psB = l0pool.tile([128, 2 * PTILE], F32, tag="l0")
                for r in range(4):
                    g = 4 * q + r
                    ps = psA if r < 2 else psB
                    dst = ps[:, ts(r % 2, PTILE)]
                    nc.tensor.matmul(dst, was[g // 4][:, ts(g % 4, 128)],
                                     pt1s[t][:], start=True, stop=False)
                for r in range(4):
                    g = 4 * q + r
                    ps = psA if r < 2 else psB
                    dst = ps[:, ts(r % 2, PTILE)]
                    nc.tensor.matmul(dst,
                                     wbs[g // 4][32 * r:32 * r + 20, ts(g % 4, 128)],
                                     pt2s[t][32 * r:32 * r + 20, :],
                                     start=False, stop=True,
                                     tile_position=(32 * r, 0))
                # drain each PSUM tile with two parallel half-acts, one per
                # engine, to cut the buffer-recycle and h0-ready latency.
                # DVE (slower) takes the earlier-finished left half.
                # Each PSUM half goes to its OWN h0 tile: L1's two
                # column-grouped matmuls then stream rhs from two distinct
                # SBUF tiles, which lets them overlap on the PE the way
                # L2's four-way column-tiled matmuls (4 distinct h1 rhs
                # tiles) demonstrably do.
                for j, ps in ((2 * q, psA), (2 * q + 1, psB)):
                    ha = h0pool.tile([128, PTILE], BF16, name="ha", tag="h0")
                    hb = h0pool.tile([128, PTILE], BF16, name="hb", tag="h0")
                    nc.vector.tensor_scalar_max(ha[:], ps[:, 0:PTILE], 0.0)
                    nc.scalar.activation(hb[:], ps[:, PTILE:], Relu)
                    h0.append((ha, hb))
            # ---- layer 1: 16 pairs of groups -> (128 = 8f*16, pix)
            h1 = []
            for p in range(16):
                ps = l12pool.tile([128, PTILE], F32, tag="l12")
                ha, hb = h0[p]
                nc.tensor.matmul(ps[0:64, :], w1s[:, ts(2 * p, 64)],
                                 ha[:], start=True, stop=True)
                nc.tensor.matmul(ps[64:128, :], w1s[:, ts(2 * p + 1, 64)],
                                 hb[:], start=True, stop=True)
                h = h1pool.tile([128, PTILE], BF16, tag="h1")
                relu(h[:], ps[:], b1s[:, p:p + 1], p)
                h1.append(h)
            # ---- layer 2: 4 blocks of 32 filters; q-major order so the 4
            # blocks' matmuls hit disjoint PE column groups concurrently
            ps2 = l12pool.tile([128, PTILE], F32, tag="l12")
            for q in range(4):
                for jj in range(4):
                    p = 4 * jj + q
                    nc.tensor.matmul(ps2[32 * jj:32 * jj + 32, :],
                                     w2s[:, ts(p, 32)], h1[p][:],
                                     start=(q == 0), stop=(q == 3),
                                     tile_position=(0, 32 * jj))
            ot = opool.tile([128, PTILE], BF16, tag="o")
            nc.scalar.activation(ot[:], ps2[:], Relu, bias=b2s[:, 0:1])
            nc.sync.dma_start(out[:, pix], ot[:])


_COMPILED = None


def _get_compiled():
    global _COMPILED
    if _COMPILED is None:
        import time as _time
        t0 = _time.time()
        nc = bacc.Bacc("TRN2", target_bir_lowering=False, debug=False,
                       num_devices=NCORES)
        with tile.TileContext(nc) as tc:
            _body(tc)
        t1 = _time.time()
        nc.compile()
        t2 = _time.time()
        print(f"[kernel] tile build+schedule {t1 - t0:.1f}s, bacc compile {t2 - t1:.1f}s",
              flush=True)
        _COMPILED = nc
    return _COMPILED


# ----------------------------------------------------------------------------
# public entry point
# ----------------------------------------------------------------------------

def kernel(x, w0, b0, w1, b1, w2, b2, _trace=False):
    x = np.asarray(x, np.float32)
    shared = _pack_weights(w0, b0, w1, b1, w2, b2)

    in_maps = []
    for k in range(NCORES):
        PT = _im2col_T(x[BPC * k:BPC * (k + 1)])
        m = dict(shared)
        m["pt1"] = PT[:128].astype(NPBF16)
        # chunk2 rows replicated at partitions 32r (4-way row tiling),
        # with a ones row at 32r+19 that carries b0 through the matmul
        pt2 = np.zeros((128, NPIX), np.float32)
        for r in range(4):
            pt2[32 * r:32 * r + 19] = PT[128:]
            pt2[32 * r + 19] = 1.0
        m["pt2"] = pt2.astype(NPBF16)
        in_maps.append(m)

    import time as _time
    nc = _get_compiled()
    t0 = _time.time()
    res = bass_utils.run_bass_kernel_spmd(
        nc, in_maps, core_ids=list(range(NCORES)), trace=_trace)
    print(f"[kernel] run_bass_kernel_spmd {_time.time() - t0:.1f}s", flush=True)

    outs = []
    for k in range(NCORES):
        oc = np.asarray(res.results[k]["out"], dtype=np.float32)  # (128, 4096)
        outs.append(oc.reshape(F, BPC, H, W).transpose(1, 2, 3, 0))
    full = np.concatenate(outs, axis=0).astype(np.float32)
    if _trace:
        return full, res
    return full


# revision 25
# speedup vs baseline: 1.0989x; 1.0717x over previous
"""Trainium2 Bass kernel for the NiN-Conv2D problem.

Network: per-pixel MLP over 7x7x3 patches, independent per filter f:
  h0 = relu(P @ W0[:,:,f] + b0)   (147 -> 32)
  h1 = relu(h0 @ W1[:,:,f] + b1)  (32 -> 16)
  out = relu(h1 @ W2[:,:,f] + b2) (16 -> 1)
for B=32, H=W=32, F=128.

Strategy: data-parallel over batch across 8 NeuronCores (4 images each).
On each core everything runs in a "feature-major" orientation: activations
live as (d*f on partitions, pixels on free dim), weights are the stationary
matmul operand, so no transposes are needed between layers.

  L0: per group of 4 filters, lhsT = W0 chunk (K=128 / K=19 accumulated),
      rhs = im2col-transposed patches (K, pix) -> PSUM (128=4f*32, pix)
  L1: per pair of groups, block-diag W1 (128, 64), two matmuls fill the
      two partition halves of one PSUM tile -> (128=8f*16, pix)
  L2: per 32-filter block, 4 accumulating block-diag matmuls -> (f, pix)

Bias+ReLU+cast(PSUM->SBUF) fused into one ACT/DVE op, split across both
engines to balance load. Matmul operands are bf16 (fp32 PSUM accumulate).

The PE cost of a matmul is ~N_free cycles regardless of K/M, so this
instruction mix sits at the bf16 structural floor; the Tile scheduler's
emergent interleaving of L1/L2 into L0 keeps the PE ~80% busy. Startup
DMAs are fine-sliced and triggered from both the sync and scalar queues
(each trigger costs ~600ns of one engine's queue) so the PE starts ~4us
earlier than with one serial trigger queue.

Host side: im2col transpose + weight packing (pure layout, no FLOPs).
"""
import numpy as np
import ml_dtypes

import concourse.bass as bass
import concourse.mybir as mybir
from concourse import bacc, tile
from concourse import bass_utils
from concourse.bass import ts

KH, KW = 7, 7
B, H, W, C, F = 32, 32, 32, 3, 128
K, D0, D1 = 147, 32, 16
NCORES = 8
BPC = B // NCORES            # 4 images per core
NPIX = BPC * H * W           # 4096 pixels per core
PTILE = 512
NT = NPIX // PTILE           # 8 pixel tiles

BF16 = mybir.dt.bfloat16
F32 = mybir.dt.float32
NPBF16 = ml_dtypes.bfloat16


# ----------------------------------------------------------------------------
# host-side packing (layout only)
# ----------------------------------------------------------------------------

def _pack_weights(w0, b0, w1, b1, w2, b2):
    """Shared (core-independent) weight/bias packing. Returns dict of np arrays."""
    w0 = np.asarray(w0, np.float32)
    w1 = np.asarray(w1, np.float32)
    w2 = np.asarray(w2, np.float32)
    b0 = np.asarray(b0, np.float32)
    b1 = np.asarray(b1, np.float32)
    b2 = np.asarray(b2, np.float32)

    w0a = np.empty((128, 32, 128), np.float32)   # [k, group, m=fl*32+d]
    # chunk2 (K rows 128..146 + bias row) packed for 4-way row-tiled
    # concurrency: group g lives at partitions 32*(g%4)+k, cols g*128+m.
    # Row 32*(g%4)+19 carries b0 (the patch tile has ones there), so the
    # PSUM result already includes the bias and the relu op needs none.
    w0b = np.zeros((128, 32, 128), np.float32)
    for g in range(32):
        m = w0[:, :, 4 * g:4 * g + 4].transpose(0, 2, 1).reshape(K, 128)
        w0a[:, g, :] = m[:128]
        r = g % 4
        w0b[32 * r:32 * r + 19, g, :] = m[128:]
        w0b[32 * r + 19, g, :] = b0[:, 4 * g:4 * g + 4].T.reshape(128)

    w1bd = np.zeros((128, 32, 64), np.float32)   # [k=fl*32+d0, g, m=fl*16+d1]
    b1s = np.empty((128, 16), np.float32)
    for g in range(32):
        for fl in range(4):
            f = 4 * g + fl
            w1bd[fl * 32:(fl + 1) * 32, g, fl * 16:(fl + 1) * 16] = w1[:, :, f]
    for p in range(16):
        for half in range(2):
            g = 2 * p + half
            b1s[half * 64:(half + 1) * 64, p] = b1[:, 4 * g:4 * g + 4].T.reshape(64)

    w2bd = np.zeros((128, 16, 32), np.float32)   # [k=half*64+fl*16+d1, pair, col]
    for p in range(16):
        for half in range(2):
            for fl in range(4):
                f = 8 * p + half * 4 + fl
                col = f - 32 * (p // 4)
                w2bd[half * 64 + fl * 16:half * 64 + (fl + 1) * 16, p, col] = w2[:, 0, f]
    b2s = b2.reshape(128, 1).astype(np.float32)

    return {
        "w0a": w0a.reshape(128, 4096).astype(NPBF16),
        "w0b": w0b.reshape(128, 4096).astype(NPBF16),
        "w1bd": w1bd.reshape(128, 2048).astype(NPBF16),
        "w2bd": w2bd.reshape(128, 512).astype(NPBF16),
        "b1s": b1s, "b2s": b2s,
    }


def _im2col_T(x_core):
    """x_core (4,32,32,3) fp32 -> PT (147, 4096) with k=(kh*7+kw)*3+c."""
    xp = np.pad(np.asarray(x_core, np.float32), ((0, 0), (3, 3), (3, 3), (0, 0)))
    PT = np.empty((K, NPIX), np.float32)
    for kh in range(KH):
        for kw in range(KW):
            blk = xp[:, kh:kh + H, kw:kw + W, :]
            t = kh * 7 + kw
            PT[t * 3:t * 3 + 3] = blk.transpose(3, 0, 1, 2).reshape(3, NPIX)
    return PT


# ----------------------------------------------------------------------------
# device kernel
# ----------------------------------------------------------------------------

def _body(tc):
    nc = tc.nc
    Relu = mybir.ActivationFunctionType.Relu
    Add, Max = mybir.AluOpType.add, mybir.AluOpType.max

    pt1 = nc.dram_tensor("pt1", [128, NPIX], BF16, kind="ExternalInput").ap()
    pt2 = nc.dram_tensor("pt2", [128, NPIX], BF16, kind="ExternalInput").ap()
    w0a = nc.dram_tensor("w0a", [128, 4096], BF16, kind="ExternalInput").ap()
    w0b = nc.dram_tensor("w0b", [128, 4096], BF16, kind="ExternalInput").ap()
    w1bd = nc.dram_tensor("w1bd", [128, 2048], BF16, kind="ExternalInput").ap()
    w2bd = nc.dram_tensor("w2bd", [128, 512], BF16, kind="ExternalInput").ap()
    b1d = nc.dram_tensor("b1s", [128, 16], F32, kind="ExternalInput").ap()
    b2d = nc.dram_tensor("b2s", [128, 1], F32, kind="ExternalInput").ap()
    out = nc.dram_tensor("out", [128, NPIX], BF16, kind="ExternalOutput").ap()

    with (
        tc.tile_pool(name="consts", bufs=1) as cpool,
        tc.tile_pool(name="h0", bufs=20) as h0pool,
        tc.tile_pool(name="h1", bufs=20) as h1pool,
        tc.tile_pool(name="outs", bufs=3) as opool,
        tc.tile_pool(name="l0p", bufs=3, space="PSUM") as l0pool,
        tc.tile_pool(name="l12p", bufs=2, space="PSUM") as l12pool,
    ):
        # ---- input staging.  Transfers are fine-sliced and triggered in
        # strict first-use order, alternating the sync/scalar trigger
        # queues, so the first quad's 4 slices (~0.5MB) land without
        # contending with later weight traffic and the PE starts ~4us
        # after the preamble instead of ~6.
        was = [cpool.tile([128, 512], BF16, name=f"w0a{i}", tag=f"w0a{i}")
               for i in range(8)]
        wbs = [cpool.tile([128, 512], BF16, name=f"w0b{i}", tag=f"w0b{i}")
               for i in range(8)]
        pt1s = [cpool.tile([128, PTILE], BF16, name=f"pt1_{t}", tag=f"pt1_{t}")
                for t in range(NT)]
        pt2s = [cpool.tile([128, PTILE], BF16, name=f"pt2_{t}", tag=f"pt2_{t}")
                for t in range(NT)]
        w1s = cpool.tile([128, 2048], BF16, name="w1", tag="w1")
        w2s = cpool.tile([128, 512], BF16, name="w2", tag="w2")
        b1s = cpool.tile([128, 16], F32, name="b1", tag="b1")
        b2s = cpool.tile([128, 1], F32, name="b2", tag="b2")

        nc.sync.dma_start(was[0][:], w0a[:, 0:512])
        nc.scalar.dma_start(pt1s[0][:], pt1[:, 0:PTILE])
        nc.scalar.dma_start(pt2s[0][:], pt2[:, 0:PTILE])
        nc.sync.dma_start(wbs[0][:], w0b[:, 0:512])
        nc.sync.dma_start(was[1][:], w0a[:, 512:1024])
        nc.scalar.dma_start(wbs[1][:], w0b[:, 512:1024])
        nc.scalar.dma_start(w1s[:], w1bd)
        nc.sync.dma_start(was[2][:], w0a[:, 1024:1536])
        nc.sync.dma_start(wbs[2][:], w0b[:, 1024:1536])
        nc.scalar.dma_start(b1s[:], b1d)
        nc.sync.dma_start(was[3][:], w0a[:, 1536:2048])
        nc.sync.dma_start(wbs[3][:], w0b[:, 1536:2048])
        nc.scalar.dma_start(w2s[:], w2bd)
        for i in range(4, 8):
            nc.sync.dma_start(was[i][:], w0a[:, ts(i, 512)])
            nc.sync.dma_start(wbs[i][:], w0b[:, ts(i, 512)])
        nc.sync.dma_start(b2s[:], b2d)
        for t in range(1, NT):
            nc.sync.dma_start(pt1s[t][:], pt1[:, ts(t, PTILE)])
            nc.sync.dma_start(pt2s[t][:], pt2[:, ts(t, PTILE)])

        def relu(dst, src, bias, idx):
            # alternate whole tiles between ScalarE and VectorE
            if idx % 2 == 0:
                nc.scalar.activation(dst, src, Relu, bias=bias)
            else:
                nc.vector.tensor_scalar(dst, src, bias, 0.0, Add, Max)

        for t in range(NT):
            pix = ts(t, PTILE)
            # ---- layer 0: 8 quads of 4 filter-groups; two (128,1024) PSUM
            # tiles per quad (2 groups each, one per column half); chunk2
            # (K rows 128..146 + bias row) runs 4-way concurrent via
            # row-group tiling. Bias rides in the matmul, so one wide
            # bias-free relu op covers a whole tile.
            h0 = []       # 16 tiles (128,1024): groups (2j, 2j+1)
            for q in range(8):
                psA = l0pool.tile([128, 2 * PTILE], F32, tag="l0")
                psB = l0pool.tile([128, 2 * PTILE], F32, tag="l0")
                for r in range(4):
                    g = 4 * q + r
                    ps = psA if r < 2 else psB
                    dst = ps[:, ts(r % 2, PTILE)]
                    nc.tensor.matmul(dst, was[g // 4][:, ts(g % 4, 128)],
                                     pt1s[t][:], start=True, stop=False)
                for r in range(4):
                    g = 4 * q + r
                    ps = psA if r < 2 else psB
                    dst = ps[:, ts(r % 2, PTILE)]
                    nc.tensor.matmul(dst,
                                     wbs[g // 4][32 * r:32 * r + 20, ts(g % 4, 128)],
                                     pt2s[t][32 * r:32 * r + 20, :],
                                     start=False, stop=True,
                                     tile_position=(32 * r, 0))
                # drain each PSUM tile with two parallel half-acts, one per
                # engine, to cut the buffer-recycle and h0-ready latency.
                # DVE (slower) takes the earlier-finished left half.
                for j, ps in ((2 * q, psA), (2 * q + 1, psB)):
                    h = h0pool.tile([128, 2 * PTILE], BF16, tag="h0")
                    if j % 2 == 0:
                        nc.scalar.activation(h[:], ps[:], Relu)
                    else:
                        nc.vector.tensor_scalar_max(h[:], ps[:], 0.0)
                    h0.append(h)
            # ---- layer 1: 16 pairs of groups -> (128 = 8f*16, pix)
            h1 = []
            for p in range(16):
                ps = l12pool.tile([128, PTILE], F32, tag="l12")
                nc.tensor.matmul(ps[0:64, :], w1s[:, ts(2 * p, 64)],
                                 h0[p][:, 0:PTILE], start=True, stop=True)
                nc.tensor.matmul(ps[64:128, :], w1s[:, ts(2 * p + 1, 64)],
                                 h0[p][:, PTILE:], start=True, stop=True)
                h = h1pool.tile([128, PTILE], BF16, tag="h1")
                relu(h[:], ps[:], b1s[:, p:p + 1], p)
                h1.append(h)
            # ---- layer 2: 4 blocks of 32 filters; q-major order so the 4
            # blocks' matmuls hit disjoint PE column groups concurrently
            ps2 = l12pool.tile([128, PTILE], F32, tag="l12")
            for q in range(4):
                for jj in range(4):
                    p = 4 * jj + q
                    nc.tensor.matmul(ps2[32 * jj:32 * jj + 32, :],
                                     w2s[:, ts(p, 32)], h1[p][:],
                                     start=(q == 0), stop=(q == 3),
                                     tile_position=(0, 32 * jj))
            ot = opool.tile([128, PTILE], BF16, tag="o")
            nc.scalar.activation(ot[:], ps2[:], Relu, bias=b2s[:, 0:1])
            nc.sync.dma_start(out[:, pix], ot[:])


_COMPILED = None


def _get_compiled():
    global _COMPILED
    if _COMPILED is None:
        import time as _time
        t0 = _time.time()
        nc = bacc.Bacc("TRN2", target_bir_lowering=False, debug=False,
                       num_devices=NCORES)
        with tile.TileContext(nc) as tc:
            _body(tc)
        t1 = _time.time()
        nc.compile()
        t2 = _time.time()
        print(f"[kernel] tile build+schedule {t1 - t0:.1f}s, bacc compile {t2 - t1:.1f}s",
              flush=True)
        _COMPILED = nc
    return _COMPILED


# ----------------------------------------------------------------------------
# public entry point
# ----------------------------------------------------------------------------

def kernel(x, w0, b0, w1, b1, w2, b2, _trace=False):
    x = np.asarray(x, np.float32)
    shared = _pack_weights(w0, b0, w1, b1, w2, b2)

    in_maps = []
    for k in range(NCORES):
        PT = _im2col_T(x[BPC * k:BPC * (k + 1)])
        m = dict(shared)
        m["pt1"] = PT[:128].astype(NPBF16)
        # chunk2 rows replicated at partitions 32r (4-way row tiling),
        # with a ones row at 32r+19 that carries b0 through the matmul
        pt2 = np.zeros((128, NPIX), np.float32)
        for r in range(4):
            pt2[32 * r:32 * r + 19] = PT[128:]
            pt2[32 * r + 19] = 1.0
        m["pt2"] = pt2.astype(NPBF16)
        in_maps.append(m)

    import time as _time
    nc = _get_compiled()
    t0 = _time.time()
    res = bass_utils.run_bass_kernel_spmd(
        nc, in_maps, core_ids=list(range(NCORES)), trace=_trace)
    print(f"[kernel] run_bass_kernel_spmd {_time.time() - t0:.1f}s", flush=True)

    outs = []
    for k in range(NCORES):
        oc = np.asarray(res.results[k]["out"], dtype=np.float32)  # (128, 4096)
        outs.append(oc.reshape(F, BPC, H, W).transpose(1, 2, 3, 0))
    full = np.concatenate(outs, axis=0).astype(np.float32)
    if _trace:
        return full, res
    return full
